# revision 2
# baseline (speedup 1.0000x reference)
"""GCN layer (GCNConv + BatchNorm + ReLU) as a distributed Bass kernel on 8 TRN2 NeuronCores.

v2 strategy (vs v1 baseline at 2.38 ms):
  - dma_gather descriptor generation was the bottleneck (one Q7 core-pair,
    ~8.5 ns/idx). Gathers now round-robin over 4 SWDGE queues -> 4 Q7 pairs
    run descgen concurrently (~2.6 ns/idx measured).
  - The Vector-engine is_equal/multiply S-matrix builds (2.2 ms) are gone:
    messages use a hybrid layout.
      * Column part: for each (dst-slot, src-range) the first H=3 messages are
        gathered into slot-aligned chunks; aggregation = plain Vector adds
        (position p of every chunk belongs to dst slot p). No S needed.
      * Leftover part: messages beyond H go into Q=2 dense S-chunks per
        (tile, range) with HOST-precomputed one-hot S matrices streamed from
        HBM (bulk DMA on otherwise idle bandwidth).
  - Contiguous node->tile assignment (node n -> tile n//128, slot n%128):
    trivial unshard, contiguous self-loop loads, uniform per-core work.
  - Per tile: agg[slot,feat] = xself + sum of 12 column chunks (Vector bf16),
    psum[feat,slot] = sum_q G_s^T S_q  (8 matmuls) + agg^T D (transpose-with-
    scale matmul, D = diag(dinv[slot]))  -> po = W @ psum -> BN stats -> ReLU.
  - BN batch stats all-reduced across the 8 cores; b cancels inside BN.
"""

import numpy as np
import ml_dtypes

import concourse.bass as bass
import concourse.bacc as bacc
import concourse.mybir as mybir
import concourse.tile as tile
from concourse.bass_utils import run_bass_kernel_spmd

N_NODES = 100000
D = 128
N_CORES = 8
TILES_PER_CORE = 98
N_TILES = N_CORES * TILES_PER_CORE  # 784
SLOTS = N_TILES * 128               # 100352 (352 pad slots)
TABLE = 25000                       # rows per gather range (int16 idx limit)
TROWS = TABLE + 1                   # +1 zero row per range
R = 4                               # src ranges
H = 3                               # column chunks per (tile, range)
Q = 2                               # S chunks per (tile, range)
CPT = H + Q                         # chunks per (tile, range) in gather stream
IDX_PER_TILE_RANGE = CPT * 128      # 640
STREAM_LEN = TILES_PER_CORE * IDX_PER_TILE_RANGE  # 62720 idxs per range stream
CH_PER_G = 8                        # chunks per dma_gather call (1024 idxs)
BN_EPS = 1e-5

BF16 = mybir.dt.bfloat16
FP32 = mybir.dt.float32
INT16 = mybir.dt.int16

LAST_RESULT = None
_BUILD_CACHE = {}


def _prep(x, edge_index):
    """Host-side packing (all numpy, O(N + E))."""
    n = N_NODES
    src = np.asarray(edge_index[0], dtype=np.int64)
    dst = np.asarray(edge_index[1], dtype=np.int64)
    e = len(src)

    deg = np.bincount(dst, minlength=n).astype(np.float32) + 1.0  # incl self loop
    dinv = (1.0 / np.sqrt(deg)).astype(np.float32)

    # gather table: 4 ranges of 25001 rows each (last row per range = zeros)
    xt = np.zeros((R * TROWS, D), dtype=ml_dtypes.bfloat16)
    xs32 = np.asarray(x, dtype=np.float32) * dinv[:, None]
    for r in range(R):
        xt[r * TROWS:r * TROWS + TABLE] = xs32[r * TABLE:(r + 1) * TABLE].astype(
            ml_dtypes.bfloat16)

    # self-loop rows: x * dinv, padded to SLOTS. The self row joins the column
    # accumulator agg, which is scaled by another dinv via the D diagonal
    # matmul -> net x * dinv^2 as required.
    xself = np.zeros((SLOTS, D), dtype=ml_dtypes.bfloat16)
    xself[:n] = xs32.astype(ml_dtypes.bfloat16)

    # D diagonal: [128 part, SLOTS] bf16, D[p, t*128+c] = (p==c) * dinv[t*128+c]
    dpad = np.zeros(SLOTS, dtype=np.float32)
    dpad[:n] = dinv
    dmat = np.zeros((128, SLOTS), dtype=ml_dtypes.bfloat16)
    pp = np.arange(SLOTS)
    dmat[pp % 128, pp] = dpad.astype(ml_dtypes.bfloat16)

    # --- message ranking per (dst, range)
    r_e = src // TABLE
    idx16 = (src % TABLE).astype(np.int64)
    key = dst * R + r_e
    order = np.argsort(key, kind="stable")
    ks = key[order]
    cnt = np.bincount(key, minlength=n * R)
    starts = np.zeros(n * R, dtype=np.int64)
    starts[1:] = np.cumsum(cnt)[:-1]
    rank = np.arange(e, dtype=np.int64) - starts[ks]  # rank within (dst, range)

    so, ro, io, do_ = src[order], r_e[order], idx16[order], dst[order]
    tile_o = do_ // 128
    slot_o = do_ % 128

    # column part: rank < H -> idx position (tile, range, h=rank, slot)
    col_idx = np.full((N_TILES, R, H, 128), TABLE, dtype=np.int16)  # zero row
    cm = rank < H
    col_idx[tile_o[cm], ro[cm], rank[cm], slot_o[cm]] = io[cm].astype(np.int16)

    # leftover part: rank >= H -> packed per (tile, range)
    lm = ~cm
    lt, lr, li, ls = tile_o[lm], ro[lm], io[lm], slot_o[lm]
    lkey = lt * R + lr
    lorder = np.argsort(lkey, kind="stable")
    lks = lkey[lorder]
    lcnt = np.bincount(lkey, minlength=N_TILES * R)
    assert lcnt.max() <= Q * 128, f"S capacity exceeded: {lcnt.max()}"
    lstarts = np.zeros(N_TILES * R, dtype=np.int64)
    lstarts[1:] = np.cumsum(lcnt)[:-1]
    lpos = np.arange(lm.sum(), dtype=np.int64) - lstarts[lks]

    s_idx = np.full((N_TILES, R, Q, 128), TABLE, dtype=np.int16)  # zero row
    s_idx[lt[lorder], lr[lorder], lpos // 128, lpos % 128] = li[lorder].astype(np.int16)
    # dense S: [tile, range, q, msg_pos(partition), dst_slot] = dinv[dst]
    s_mat = np.zeros((N_TILES, R, Q, 128, 128), dtype=ml_dtypes.bfloat16)
    s_mat[lt[lorder], lr[lorder], lpos // 128, lpos % 128,
          ls[lorder]] = dinv[do_[lm][lorder]].astype(ml_dtypes.bfloat16)

    # --- per-core gather streams + idx wrap
    idxs, smats, dmats, xselfs = [], [], [], []
    for k in range(N_CORES):
        t0, t1 = k * TILES_PER_CORE, (k + 1) * TILES_PER_CORE
        streams = []
        for r in range(R):
            # per tile: [col h0, col h1, col h2, S q0, S q1] each 128
            st = np.concatenate(
                [col_idx[t0:t1, r].reshape(TILES_PER_CORE, H * 128),
                 s_idx[t0:t1, r].reshape(TILES_PER_CORE, Q * 128)], axis=1
            ).reshape(-1)  # [STREAM_LEN]
            assert st.shape[0] == STREAM_LEN
            wrapped = st.reshape(STREAM_LEN // 16, 16).T  # [16, len/16]
            streams.append(np.tile(wrapped, (8, 1)))      # [128, len/16]
        idxs.append(np.ascontiguousarray(np.concatenate(streams, axis=1)))
        # S per core: [128 msg-part, TPC * R * Q * 128] ordered (t, r, q)
        sm = s_mat[t0:t1].transpose(3, 0, 1, 2, 4).reshape(128, -1)
        smats.append(np.ascontiguousarray(sm))
        dmats.append(np.ascontiguousarray(dmat[:, t0 * 128:t1 * 128]))
        # xself staged slot-major: [128 slot-part, TPC * D]
        xs_core = xself[t0 * 128:t1 * 128].reshape(TILES_PER_CORE, 128, D)
        xselfs.append(np.ascontiguousarray(
            xs_core.transpose(1, 0, 2).reshape(128, TILES_PER_CORE * D)))

    return dict(xt=xt, idxs=idxs, smats=smats, dmats=dmats, xselfs=xselfs)


def _build():
    """Build the SPMD Bass program (identical across cores)."""
    nc = bacc.Bacc(None, num_devices=N_CORES, num_swdge_queues=4)

    n_calls = -(-STREAM_LEN // (CH_PER_G * 128))  # gather calls per range (62)
    scols = TILES_PER_CORE * R * Q * 128          # S columns per core

    xt_d = nc.dram_tensor("xt", [R * TROWS, D], BF16, kind="ExternalInput")
    idx_d = nc.dram_tensor("idx", [128, R * STREAM_LEN // 16], INT16,
                           kind="ExternalInput")
    s_d = nc.dram_tensor("smat", [128, scols], BF16, kind="ExternalInput")
    dm_d = nc.dram_tensor("dmat", [128, TILES_PER_CORE * 128], BF16,
                          kind="ExternalInput")
    xs_d = nc.dram_tensor("xself", [128, TILES_PER_CORE * D], BF16,
                          kind="ExternalInput")
    wt_d = nc.dram_tensor("wt", [D, D], BF16, kind="ExternalInput")
    gb_d = nc.dram_tensor("gb", [128, 2], FP32, kind="ExternalInput")
    out_d = nc.dram_tensor("out", [128, TILES_PER_CORE * 128], FP32,
                           kind="ExternalOutput")

    cc_in = nc.dram_tensor("cc_in", [128, 2], FP32)
    cc_out = nc.dram_tensor("cc_out", [128, 2], FP32, addr_space="Shared")

    AF = mybir.ActivationFunctionType
    ALU = mybir.AluOpType
    AX = mybir.AxisListType

    with tile.TileContext(nc) as tc:
        with (
            tc.tile_pool(name="const", bufs=1) as cpool,
            tc.tile_pool(name="gbuf", bufs=8) as gpool,
            tc.tile_pool(name="sbuf", bufs=3) as spool,
            tc.tile_pool(name="aggb", bufs=3) as apool,
            tc.tile_pool(name="small", bufs=2) as smpool,
            tc.tile_pool(name="pagg", bufs=3, space="PSUM") as pagg_pool,
            tc.tile_pool(name="pout", bufs=2, space="PSUM") as pout_pool,
        ):
            idx_sb = cpool.tile([128, R * STREAM_LEN // 16], INT16, tag="idx")
            nc.sync.dma_start(out=idx_sb[:], in_=idx_d[:])
            dm_sb = cpool.tile([128, TILES_PER_CORE * 128], BF16, tag="dmat")
            nc.sync.dma_start(out=dm_sb[:], in_=dm_d[:])
            xsel_sb = cpool.tile([128, TILES_PER_CORE * D], BF16, tag="xsel")
            nc.sync.dma_start(out=xsel_sb[:], in_=xs_d[:])
            wt_sb = cpool.tile([128, D], BF16, tag="wt")
            nc.sync.dma_start(out=wt_sb[:], in_=wt_d[:])
            gb_sb = cpool.tile([128, 2], FP32, tag="gb")
            nc.sync.dma_start(out=gb_sb[:], in_=gb_d[:])

            pre_bn = cpool.tile([128, TILES_PER_CORE * 128], FP32, tag="prebn")
            sum_sl = cpool.tile([128, TILES_PER_CORE], FP32, tag="sumsl")
            sq_sl = cpool.tile([128, TILES_PER_CORE], FP32, tag="sqsl")

            g_tiles = {}

            def get_G(r, call):
                if (r, call) not in g_tiles:
                    nch = min(CH_PER_G, -(-STREAM_LEN // 128) - call * CH_PER_G)
                    ni = nch * 128
                    G = gpool.tile([128, nch * D], BF16, tag=f"G{r}")
                    base = r * STREAM_LEN + call * CH_PER_G * 128
                    nc.gpsimd.dma_gather(
                        out_ap=G[:].rearrange("p (c f) -> p c f", f=D),
                        in_ap=xt_d[r * TROWS:(r + 1) * TROWS, :],
                        idxs_ap=idx_sb[:, base // 16:(base + ni) // 16],
                        num_idxs=ni,
                        num_idxs_reg=ni,
                        elem_size=D,
                        queue_num=r,
                    )
                    g_tiles[(r, call)] = G
                return g_tiles[(r, call)]

            def gslice(r, chunk):
                G = get_G(r, chunk // CH_PER_G)
                c = chunk % CH_PER_G
                return G[:, c * D:(c + 1) * D]

            for t in range(TILES_PER_CORE):
                # ---- column part: agg[slot, feat] = xself + sum of 12 chunks
                agg = apool.tile([128, D], BF16, tag="agg")
                first = True
                for r in range(R):
                    for h in range(H):
                        chunk = t * CPT + h
                        g = gslice(r, chunk)
                        if first:
                            nc.vector.tensor_tensor(
                                out=agg[:], in0=xsel_sb[:, t * D:(t + 1) * D],
                                in1=g, op=ALU.add)
                            first = False
                        else:
                            nc.vector.tensor_tensor(
                                out=agg[:], in0=agg[:], in1=g, op=ALU.add)

                # ---- psum[feat, slot]: S chunks then transpose-with-scale
                pa = pagg_pool.tile([128, 128], FP32, tag="pa")
                St = spool.tile([128, R * Q * 128], BF16, tag="S")
                nc.sync.dma_start(
                    out=St[:], in_=s_d[:, t * R * Q * 128:(t + 1) * R * Q * 128])
                m = 0
                for r in range(R):
                    for q in range(Q):
                        chunk = t * CPT + H + q
                        sc = (r * Q + q) * 128
                        nc.tensor.matmul(
                            pa[:], lhsT=gslice(r, chunk), rhs=St[:, sc:sc + 128],
                            start=(m == 0), stop=False,
                        )
                        m += 1
                nc.tensor.matmul(
                    pa[:], lhsT=agg[:], rhs=dm_sb[:, t * 128:(t + 1) * 128],
                    start=False, stop=True,
                )

                tr = apool.tile([128, 128], BF16, tag="tr")
                nc.vector.tensor_copy(out=tr[:], in_=pa[:])
                po = pout_pool.tile([128, 128], FP32, tag="po")
                nc.tensor.matmul(po[:], lhsT=wt_sb[:], rhs=tr[:],
                                 start=True, stop=True)

                nc.vector.tensor_reduce(
                    out=sum_sl[:, t:t + 1], in_=po[:], axis=AX.X, op=ALU.add)
                sq = spool.tile([128, 128], FP32, tag="sq")
                nc.scalar.activation(
                    out=sq[:], in_=po[:], func=AF.Square,
                    accum_out=sq_sl[:, t:t + 1],
                )
                nc.vector.tensor_copy(
                    out=pre_bn[:, t * 128:(t + 1) * 128], in_=po[:])

            # ---- BN stats: local reduce, all-reduce, scale/shift
            stats = smpool.tile([128, 2], FP32, tag="stats")
            nc.vector.tensor_reduce(out=stats[:, 0:1], in_=sum_sl[:], axis=AX.X,
                                    op=ALU.add)
            nc.vector.tensor_reduce(out=stats[:, 1:2], in_=sq_sl[:], axis=AX.X,
                                    op=ALU.add)
            nc.sync.dma_start(out=cc_in[:], in_=stats[:])
            nc.gpsimd.collective_compute(
                "AllReduce", ALU.add,
                replica_groups=[list(range(N_CORES))],
                ins=[cc_in[:]], outs=[cc_out[:]],
            )
            statg = smpool.tile([128, 2], FP32, tag="statg")
            nc.sync.dma_start(out=statg[:], in_=cc_out[:])

            mean = smpool.tile([128, 1], FP32, tag="mean")
            nc.vector.tensor_scalar_mul(mean[:], statg[:, 0:1], 1.0 / N_NODES)
            ex2 = smpool.tile([128, 1], FP32, tag="ex2")
            nc.vector.tensor_scalar_mul(ex2[:], statg[:, 1:2], 1.0 / N_NODES)
            m2 = smpool.tile([128, 1], FP32, tag="m2")
            nc.vector.tensor_tensor(out=m2[:], in0=mean[:], in1=mean[:], op=ALU.mult)
            var = smpool.tile([128, 1], FP32, tag="var")
            nc.vector.tensor_tensor(out=var[:], in0=ex2[:], in1=m2[:], op=ALU.subtract)
            nc.vector.tensor_scalar_add(var[:], var[:], BN_EPS)
            inv = smpool.tile([128, 1], FP32, tag="inv")
            nc.vector.reciprocal(inv[:], var[:])
            istd = smpool.tile([128, 1], FP32, tag="istd")
            nc.scalar.sqrt(istd[:], inv[:])
            scale = smpool.tile([128, 1], FP32, tag="scale")
            nc.vector.tensor_tensor(out=scale[:], in0=gb_sb[:, 0:1], in1=istd[:],
                                    op=ALU.mult)
            msc = smpool.tile([128, 1], FP32, tag="msc")
            nc.vector.tensor_tensor(out=msc[:], in0=mean[:], in1=scale[:], op=ALU.mult)
            shift = smpool.tile([128, 1], FP32, tag="shift")
            nc.vector.tensor_tensor(out=shift[:], in0=gb_sb[:, 1:2], in1=msc[:],
                                    op=ALU.subtract)

            for t in range(TILES_PER_CORE):
                nc.scalar.activation(
                    out=pre_bn[:, t * 128:(t + 1) * 128],
                    in_=pre_bn[:, t * 128:(t + 1) * 128],
                    func=AF.Relu, scale=scale[:], bias=shift[:],
                )
            nc.sync.dma_start(out=out_d[:], in_=pre_bn[:])

    nc.compile()
    return nc


def _get_program():
    if "v2" not in _BUILD_CACHE:
        _BUILD_CACHE["v2"] = _build()
    return _BUILD_CACHE["v2"]


def kernel(x, edge_index, W, b, gamma, beta, _run_fn=None):
    x = np.asarray(x, dtype=np.float32)
    edge_index = np.asarray(edge_index)
    W = np.asarray(W, dtype=np.float32)
    gamma = np.asarray(gamma, dtype=np.float32)
    beta = np.asarray(beta, dtype=np.float32)

    n = x.shape[0]
    assert n == N_NODES and x.shape[1] == D

    plan = _prep(x, edge_index)

    wt = np.ascontiguousarray(W.T.astype(ml_dtypes.bfloat16))  # [in_f, out_o]
    gb = np.stack([gamma, beta], axis=1).astype(np.float32)

    in_maps = []
    for k in range(N_CORES):
        in_maps.append(dict(
            xt=plan["xt"], idx=plan["idxs"][k], smat=plan["smats"][k],
            dmat=plan["dmats"][k], xself=plan["xselfs"][k],
            wt=wt, gb=gb,
        ))

    nc = _get_program()

    global LAST_RESULT
    if _run_fn is not None:
        results = _run_fn(nc, in_maps)
    else:
        LAST_RESULT = run_bass_kernel_spmd(nc, in_maps, core_ids=list(range(N_CORES)))
        results = LAST_RESULT.results

    # ---- unshard: out[k] is [128 feat, TPC*128 slots]; nodes are contiguous
    y = np.empty((n, D), dtype=np.float32)
    per = TILES_PER_CORE * 128
    for k in range(N_CORES):
        lo = k * per
        hi = min(n, lo + per)
        yk = np.asarray(results[k]["out"], dtype=np.float32)
        y[lo:hi] = yk[:, :hi - lo].T
    return y
